# revision 1
# baseline (speedup 1.0000x reference)
"""Trainium2 Bass kernel: causal multi-head self-attention (B=2, T=2048, C=1024, H=16).

Sharding: 8 NeuronCores = 2-way data parallel (batch) x 4-way tensor parallel
(head groups of 4 heads). Each core computes, for its batch b and heads
[4g, 4g+4):
    qkvT = (x_b @ W_attn[:, cols]).T      (via transposed-x layout)
    attention (causal, flash-style, un-normalized accumulation + ones-column
    denominators, softmax without max-subtraction -- scores are O(5))
    partial out.T = (y_heads @ W_proj[rows]).T
The host pre-transposes/casts inputs, and sums the 4 tensor-parallel partial
outputs per batch + transposes back. No on-device collectives, no on-device
transposes.
"""

import os
from contextlib import ExitStack

import ml_dtypes
import numpy as np

import concourse.bass as bass
import concourse.tile as tile
from concourse import bacc, mybir
from concourse.bass_utils import run_bass_kernel_spmd

# Problem dims (hardcoded per the harness contract).
B, T, C, H = 2, 2048, 1024, 16
D = C // H          # 64 head dim
NCORES = 8
TP = 4              # tensor-parallel head groups
HG = H // TP        # 4 heads per core
CH = HG * D         # 256 channels per core
P = 128
CK = C // P         # 8 contraction tiles over C
NKT = T // P        # 16 key tiles
TQ = 512            # query chunk (one PSUM bank of fp32)
NQC = T // TQ       # 4 query chunks
DTQ = TQ // P       # 4 key tiles per query chunk (diagonal span)
BF = mybir.dt.bfloat16
F32 = mybir.dt.float32
SCALE = 1.0 / float(np.sqrt(D))

_USE_GPSIMD_BCAST = os.environ.get("KBENCH_GPSIMD_BCAST", "1") == "1"
_DMA_FROM_PSUM = os.environ.get("KBENCH_DMA_FROM_PSUM", "0") == "1"


def _body(nc, tc, ctx, xT, wqk, wv, wp, outT):
    Exp = mybir.ActivationFunctionType.Exp

    const = ctx.enter_context(tc.tile_pool(name="const", bufs=1))
    wpool = ctx.enter_context(tc.tile_pool(name="wpool", bufs=1))
    xpool = ctx.enter_context(tc.tile_pool(name="xpool", bufs=1))
    qkpool = ctx.enter_context(tc.tile_pool(name="qkpool", bufs=1))
    vpool = ctx.enter_context(tc.tile_pool(name="vpool", bufs=1))
    ypool = ctx.enter_context(tc.tile_pool(name="ypool", bufs=2))
    etpool = ctx.enter_context(tc.tile_pool(name="etpool", bufs=4))
    rpool = ctx.enter_context(tc.tile_pool(name="rpool", bufs=3))
    opool = ctx.enter_context(tc.tile_pool(name="opool", bufs=3))
    mm_ps = ctx.enter_context(tc.tile_pool(name="mm_ps", bufs=2, space="PSUM"))
    s_ps = ctx.enter_context(tc.tile_pool(name="s_ps", bufs=2, space="PSUM"))
    z_ps = ctx.enter_context(tc.tile_pool(name="z_ps", bufs=2, space="PSUM"))

    # Constants: causal mask tile (keep where q-offset >= k-offset) and ones row.
    mask = const.tile([P, TQ], BF, tag="mask", name="mask")
    nc.gpsimd.memset(mask, 1.0)
    nc.gpsimd.affine_select(
        out=mask,
        in_=mask,
        compare_op=mybir.AluOpType.is_ge,
        fill=0.0,
        base=0,
        pattern=[[1, TQ]],
        channel_multiplier=-1,
    )
    ones = const.tile([1, D], BF, tag="ones", name="ones")
    nc.vector.memset(ones, 1.0)

    # Loads, in need-order: (wqk_ci | xT_ci first half) interleaved feeds the
    # first qk matmul group ASAP; wv before the chunk-0 v units; xT second
    # halves are not needed until chunk 2; wp last (projection).
    xT_sb = [xpool.tile([P, T], BF, tag=f"x{ci}", name=f"x{ci}") for ci in range(CK)]
    wqk_sb = [
        wpool.tile([P, 2 * CH], BF, tag=f"wqk{ci}", name=f"wqk{ci}") for ci in range(CK)
    ]
    wv_sb = [wpool.tile([P, CH], BF, tag=f"wv{ci}", name=f"wv{ci}") for ci in range(CK)]
    wp_sb = [
        wpool.tile([P, C], BF, tag=f"wp{ck}", name=f"wp{ck}") for ck in range(CH // P)
    ]
    for ci in range(CK):
        nc.sync.dma_start(wqk_sb[ci], wqk[ci * P : (ci + 1) * P, :])
        nc.sync.dma_start(xT_sb[ci][:, 0 : T // 2], xT[ci * P : (ci + 1) * P, 0 : T // 2])
    for ci in range(CK):
        nc.sync.dma_start(wv_sb[ci], wv[ci * P : (ci + 1) * P, :])
    for ci in range(CK):
        nc.sync.dma_start(xT_sb[ci][:, T // 2 : T], xT[ci * P : (ci + 1) * P, T // 2 : T])
    for ck in range(CH // P):
        nc.sync.dma_start(wp_sb[ck], wp[ck * P : (ck + 1) * P, :])

    # qkT rows: m=0 -> q heads 0,1; m=1 -> q heads 2,3; m=2 -> k heads 0,1; m=3 -> k heads 2,3
    qk_sb = [qkpool.tile([P, T], BF, tag=f"qk{m}", name=f"qk{m}") for m in range(4)]
    v_sb = [vpool.tile([P, HG * (D + 1)], BF, tag=f"v{tm}", name=f"v{tm}") for tm in range(NKT)]

    # ---------- work-unit generators (closures emit instructions) ----------

    def p1_qk_unit(m, j):
        def emit():
            ps = mm_ps.tile([P, TQ], F32, tag="mm", name="mm")
            for ci in range(CK):
                nc.tensor.matmul(
                    ps,
                    lhsT=wqk_sb[ci][:, m * P : (m + 1) * P],
                    rhs=xT_sb[ci][:, j * TQ : (j + 1) * TQ],
                    start=(ci == 0),
                    stop=(ci == CK - 1),
                )
            nc.vector.tensor_copy(qk_sb[m][:, j * TQ : (j + 1) * TQ], ps)
        return emit

    def p1_v_unit(tm):
        def emit():
            vt = v_sb[tm]
            vt3 = vt.rearrange("p (h u) -> p h u", h=HG)
            nc.vector.memset(vt3[:, :, D : D + 1], 1.0)  # ones columns
            ps = mm_ps.tile([P, CH], F32, tag="mm", name="mm")
            for ci in range(CK):
                nc.tensor.matmul(
                    ps,
                    lhsT=xT_sb[ci][:, tm * P : (tm + 1) * P],
                    rhs=wv_sb[ci],
                    start=(ci == 0),
                    stop=(ci == CK - 1),
                )
            nc.vector.tensor_copy(vt3[:, :, 0:D], ps.rearrange("p (h u) -> p h u", h=HG))
        return emit

    def p1_units(j):
        us = [p1_qk_unit(m, j) for m in range(4)]
        us += [p1_v_unit(tm) for tm in range(DTQ * j, DTQ * (j + 1))]
        return us

    def attn_units(qc, hp, yts):
        """Software-pipelined: s/exp for kt+1 issued before z for kt."""
        qT = qk_sb[hp]
        kT = qk_sb[2 + hp]
        nkt_q = DTQ * (qc + 1)
        state = {}

        def emit_s(kt):
            m = kt - DTQ * qc
            col0 = m * P if m > 0 else 0
            w = TQ - col0
            sp = s_ps.tile([P, 2 * TQ], F32, tag="sps", name="sps")
            nc.tensor.matmul(
                sp[:, col0:TQ],
                lhsT=kT[0:D, kt * P : (kt + 1) * P],
                rhs=qT[0:D, qc * TQ + col0 : (qc + 1) * TQ],
                start=True,
                stop=True,
            )
            nc.tensor.matmul(
                sp[:, TQ + col0 : 2 * TQ],
                lhsT=kT[D : 2 * D, kt * P : (kt + 1) * P],
                rhs=qT[D : 2 * D, qc * TQ + col0 : (qc + 1) * TQ],
                start=True,
                stop=True,
            )
            ep = etpool.tile([P, 2 * TQ], BF, tag="et", name="et")
            sp3 = sp.rearrange("p (i u) -> p i u", i=2)
            ep3 = ep.rearrange("p (i u) -> p i u", i=2)
            nc.scalar.activation(ep3[:, :, col0:TQ], sp3[:, :, col0:TQ], Exp, scale=SCALE)
            if m >= 0:
                # The mask differs from 1.0 only in the first P columns of the
                # valid rectangle (the diagonal triangle f - col0 < p).
                mw = min(P, w)
                nc.vector.tensor_mul(
                    ep[:, col0 : col0 + mw], ep[:, col0 : col0 + mw], mask[:, 0:mw]
                )
                nc.vector.tensor_mul(
                    ep[:, TQ + col0 : TQ + col0 + mw],
                    ep[:, TQ + col0 : TQ + col0 + mw],
                    mask[:, 0:mw],
                )
            state[kt] = ep

        def z_unit(kt):
            def emit():
                if kt == 0:
                    emit_s(0)
                if kt + 1 < nkt_q:
                    emit_s(kt + 1)
                m = kt - DTQ * qc
                col0 = m * P if m > 0 else 0
                ep = state.pop(kt)
                last = kt == nkt_q - 1
                for zi in range(2):
                    h = 2 * hp + zi
                    nc.tensor.matmul(
                        state["z"][zi][:, col0:TQ],
                        lhsT=v_sb[kt][:, h * (D + 1) : (h + 1) * (D + 1)],
                        rhs=ep[:, zi * TQ + col0 : zi * TQ + TQ],
                        start=(kt == 0),
                        stop=last,
                    )
            return emit

        def alloc_z():
            state["z"] = (
                z_ps.tile([D + 1, TQ], F32, tag="z", name="z"),
                z_ps.tile([D + 1, TQ], F32, tag="z", name="z"),
            )

        def norm_unit():
            def emit():
                # One fast PSUM->SBUF copy per z releases its PSUM slot for the
                # next head pair; the reciprocal/broadcast/mul chain then runs
                # from SBUF off the critical path.
                for zi, z in enumerate(state["z"]):
                    zc = rpool.tile([D + 1, TQ], F32, tag="zc", name="zc")
                    nc.vector.tensor_copy(zc, z)
                    r32 = rpool.tile([1, TQ], F32, tag="r32", name="r32")
                    nc.vector.reciprocal(r32, zc[D : D + 1, :])
                    rb = rpool.tile([D, TQ], F32, tag="rb", name="rb")
                    if _USE_GPSIMD_BCAST:
                        nc.gpsimd.partition_broadcast(rb, r32)
                    else:
                        r16 = rpool.tile([1, TQ], BF, tag="r16", name="r16")
                        nc.vector.tensor_copy(r16, r32)
                        rbp = s_ps.tile([D, TQ], F32, tag="sps", name="sps")
                        nc.tensor.matmul(rbp, lhsT=ones, rhs=r16, start=True, stop=True)
                        nc.vector.tensor_copy(rb, rbp)
                    nc.vector.tensor_mul(
                        yts[hp][zi * D : (zi + 1) * D, :], zc[0:D, :], rb
                    )
            return emit

        units = []

        def first():
            alloc_z()
            z_unit(0)()
        units.append(first)
        units += [z_unit(kt) for kt in range(1, nkt_q)]
        units.append(norm_unit())
        return units

    def proj_units(qc, yts):
        us = []
        for cm in range(CK):
            def emit(cm=cm):
                ps = mm_ps.tile([P, TQ], F32, tag="mm", name="mm")
                for ck in range(CH // P):
                    nc.tensor.matmul(
                        ps,
                        lhsT=wp_sb[ck][:, cm * P : (cm + 1) * P],
                        rhs=yts[ck],
                        start=(ck == 0),
                        stop=(ck == CH // P - 1),
                    )
                dst = outT[cm * P : (cm + 1) * P, qc * TQ : (qc + 1) * TQ]
                ot = opool.tile([P, TQ], F32, tag="ot", name="ot")
                nc.vector.tensor_copy(ot, ps)
                nc.sync.dma_start(dst, ot)
            us.append(emit)
        return us

    # ---------- emission ----------
    # Interleave into each chunk's ACT-paced attention stream: the next
    # chunk's phase-1 matmul groups and the PREVIOUS chunk's projection (so
    # the last chunk still has PE fill work).
    for u in p1_units(0):
        u()
    prev_proj = []
    for qc in range(NQC):
        yts = [ypool.tile([P, TQ], BF, tag=f"y{hp}", name=f"y{hp}") for hp in range(2)]
        steps = attn_units(qc, 0, yts) + attn_units(qc, 1, yts)
        fill = prev_proj + (p1_units(qc + 1) if qc + 1 < NQC else [])
        if fill:
            stride = max(1, len(steps) // len(fill))
            out_steps = []
            fi = 0
            for i, st in enumerate(steps):
                out_steps.append(st)
                if i % stride == stride - 1 and fi < len(fill):
                    out_steps.append(fill[fi])
                    fi += 1
            out_steps += fill[fi:]
            steps = out_steps
        for st in steps:
            st()
        prev_proj = proj_units(qc, yts)
    for u in prev_proj:
        u()


def build_program(reps=1):
    nc = bacc.Bacc(
        "TRN2",
        debug=False,
        enable_asserts=False,
        target_bir_lowering=False,
        num_devices=NCORES,
    )
    xT = nc.dram_tensor("xT", [C, T], BF, kind="ExternalInput").ap()
    wqk = nc.dram_tensor("wqk", [C, 2 * CH], BF, kind="ExternalInput").ap()
    wv = nc.dram_tensor("wv", [C, CH], BF, kind="ExternalInput").ap()
    wp = nc.dram_tensor("wp", [CH, C], BF, kind="ExternalInput").ap()
    outT = nc.dram_tensor("outT", [C, T], F32, kind="ExternalOutput").ap()
    with tile.TileContext(nc) as tc:
        for _ in range(reps):
            with ExitStack() as ctx:
                _body(nc, tc, ctx, xT, wqk, wv, wp, outT)
    nc.compile()
    return nc


_PROGRAM = None


def _get_program():
    global _PROGRAM
    if _PROGRAM is None:
        _PROGRAM = build_program()
    return _PROGRAM


def make_in_maps(x, W_attn, W_proj):
    bf = ml_dtypes.bfloat16
    x = np.asarray(x, dtype=np.float32)
    W_attn = np.asarray(W_attn, dtype=np.float32)
    W_proj = np.asarray(W_proj, dtype=np.float32)
    in_maps = []
    for core in range(NCORES):
        b, g = divmod(core, TP)
        xT_b = np.ascontiguousarray(x[b].T).astype(bf)
        wq = W_attn[:, g * CH : (g + 1) * CH]
        wk = W_attn[:, C + g * CH : C + (g + 1) * CH]
        wqk = np.concatenate([wq, wk], axis=1).astype(bf)
        wv = np.ascontiguousarray(W_attn[:, 2 * C + g * CH : 2 * C + (g + 1) * CH]).astype(bf)
        wp = np.ascontiguousarray(W_proj[g * CH : (g + 1) * CH, :]).astype(bf)
        in_maps.append({"xT": xT_b, "wqk": wqk, "wv": wv, "wp": wp})
    return in_maps


def kernel(x, W_attn, W_proj):
    nc = _get_program()
    in_maps = make_in_maps(x, W_attn, W_proj)
    want_trace = os.environ.get("KBENCH_TRACE", "0") == "1"
    if want_trace:
        try:  # NTFF profiling hook is absent in some containers
            from antenv.axon_hooks import get_axon_ntff_profile_hook  # noqa: F401
        except ImportError:
            want_trace = False
    res = run_bass_kernel_spmd(
        nc,
        in_maps,
        core_ids=list(range(NCORES)),
        trace=want_trace,
    )
    kernel.last_results = res
    outs = [r["outT"] for r in res.results]
    out = np.empty((B, T, C), dtype=np.float32)
    for b in range(B):
        acc = outs[TP * b]
        for g in range(1, TP):
            acc = acc + outs[TP * b + g]
        out[b] = acc.T
    return out

